# revision 1
# baseline (speedup 1.0000x reference)
"""Trainium2 Bass kernel for nn_JambaAttentionDecoderLayer (8-core SPMD).

Sharding: tensor-parallel attention (2 q-heads + 1 kv-head per core,
o-proj column-sharded, two AllGathers) + expert parallelism for the MoE
(1 expert per core, dense over tokens, ReduceScatter combine).

Everything on-device is computed in a feature-major ("transposed") layout
[feature, token] so every matmul contraction dim lands on SBUF partitions
without runtime transposes of activations.  Weights are transposed/packed
on the host while sharding.  Big matmuls run in float32r (full PE speed
for free-dim >= 256, ~2^-13 rounding).
"""

import numpy as np

import concourse.bass as bass
import concourse.tile as tile
import concourse.mybir as mybir
from concourse import bacc
from concourse.bass_utils import run_bass_kernel_spmd

# dims (hardcoded per spec)
T = 1024
H = 1024
NH = 16
NKV = 4
HD = 64
I = 2816
E = 8
SW = 512
EPS = 1e-6
SCALE = HD ** -0.5

NCORES = 8
P = 128
KT = H // P          # 8 k-tiles over H
JT = I // P          # 22 k-tiles over I
MT = H // P          # 8 m-tiles over H
NEG = -1.0e30

f32 = mybir.dt.float32
f32r = mybir.dt.float32r

# attention mask offsets: off = q_tile_start - k_tile_start for [128k,512q] tiles
OFFS = [-384, -256, -128, 0, 128, 256, 384, 512]
QT_KIS = {0: list(range(0, 4)), 1: list(range(0, 8))}

AxX = mybir.AxisListType.X
Alu = mybir.AluOpType
Act = mybir.ActivationFunctionType


def _build(profile=False):
    ndev = 1 if profile else NCORES
    nc = bacc.Bacc("TRN2", target_bir_lowering=False, debug=False,
                   num_devices=ndev)

    # ---- kernel I/O ----
    hT_d = nc.dram_tensor("hT", [H, T], f32, kind="ExternalInput")
    qkvwT_d = nc.dram_tensor("qkvwT", [H, 256], f32r, kind="ExternalInput")
    owT_d = nc.dram_tensor("owT", [H, P], f32, kind="ExternalInput")
    ln1w_d = nc.dram_tensor("ln1w", [P, KT], f32, kind="ExternalInput")
    ln2w_d = nc.dram_tensor("ln2w", [P, KT], f32, kind="ExternalInput")
    rwT_d = nc.dram_tensor("rwT", [P, KT, E], f32r, kind="ExternalInput")
    wsp_d = nc.dram_tensor("wsp", [2 * JT, P, KT * P], f32r, kind="ExternalInput")
    w2p_d = nc.dram_tensor("w2p", [MT, P, JT * P], f32r, kind="ExternalInput")
    amask_d = nc.dram_tensor("amask", [len(OFFS), P, 512], mybir.dt.bfloat16,
                             kind="ExternalInput")
    onehot_d = nc.dram_tensor("onehot", [E, 1], f32r, kind="ExternalInput")
    ones128_d = nc.dram_tensor("ones128", [P, 1], f32r, kind="ExternalInput")
    ones1r_d = nc.dram_tensor("ones1r", [1, P], f32r, kind="ExternalInput")

    moe_sl_d = nc.dram_tensor("moe_slice", [P, T], f32, kind="ExternalOutput")
    residT_d = nc.dram_tensor("residT", [H, T], f32, kind="ExternalOutput")

    rg = [list(range(NCORES))]

    import contextlib
    lp = getattr(nc, "allow_low_precision", None)
    lp_cm = lp(reason="float32r matmul operands; rounding ~2^-13 acceptable") \
        if lp else contextlib.nullcontext()
    with lp_cm, tile.TileContext(nc) as tc:
        with tc.tile_pool(name="const", bufs=1) as cpool, \
             tc.tile_pool(name="persist", bufs=1) as pers, \
             tc.tile_pool(name="dram", bufs=1, space="DRAM") as dram:

            # ---- constants ----
            ones128 = cpool.tile([P, 1], f32r)
            nc.sync.dma_start(ones128[:], ones128_d[:])
            ones1r = cpool.tile([1, P], f32r)
            nc.sync.dma_start(ones1r[:], ones1r_d[:])
            onehot = cpool.tile([E, 1], f32r)
            nc.sync.dma_start(onehot[:], onehot_d[:])
            ln1w = cpool.tile([P, KT], f32)
            nc.sync.dma_start(ln1w[:], ln1w_d[:])
            ln2w = cpool.tile([P, KT], f32)
            nc.sync.dma_start(ln2w[:], ln2w_d[:])
            ident = cpool.tile([P, P], f32)
            from concourse.masks import make_identity
            make_identity(nc, ident[:])

            # dram bounce buffers for collectives
            ag1_in = dram.tile([P, T], f32)
            ag1_out = dram.tile([H, T], f32, addr_space="Shared")
            ag2_in = dram.tile([P, T], f32)
            ag2_out = dram.tile([H, T], f32, addr_space="Shared")
            rs_in = dram.tile([H, T], f32)
            rs_out = dram.tile([P, T], f32)

            # =========== RMSNorm helper (feature-major) ===========
            def rmsnorm(src_tile, lnw_tile, dst_tile):
                with tc.tile_pool(name="rn", bufs=1) as tmp, \
                     tc.tile_pool(name="rnps", bufs=1, space="PSUM") as psum:
                    vs = [None, None]
                    for ni in range(2):
                        pv = psum.tile([1, 512], f32, tag="pvar")
                        for k in range(KT):
                            sq = tmp.tile([P, 512], f32r, tag="sq", bufs=2)
                            nc.scalar.activation(
                                sq[:], src_tile[:, k, ni * 512:(ni + 1) * 512],
                                Act.Square)
                            nc.tensor.matmul(pv[:], ones128[:], sq[:],
                                             start=(k == 0), stop=(k == KT - 1))
                        v = tmp.tile([1, 512], f32, tag="vv")
                        nc.vector.tensor_scalar(v[:], pv[:], 1.0 / H, EPS,
                                                Alu.mult, Alu.add)
                        sd = tmp.tile([1, 512], f32, tag="sd")
                        nc.scalar.activation(sd[:], v[:], Act.Sqrt)
                        s = tmp.tile([1, 512], f32r, tag="ss")
                        nc.vector.reciprocal(s[:], sd[:])
                        pb = psum.tile([P, 512], f32, tag="pbc", bufs=2)
                        nc.tensor.matmul(pb[:], ones1r[:], s[:],
                                         start=True, stop=True)
                        vs[ni] = pb
                    for ni in range(2):
                        for k in range(KT):
                            nc.vector.scalar_tensor_tensor(
                                dst_tile[:, k, ni * 512:(ni + 1) * 512],
                                src_tile[:, k, ni * 512:(ni + 1) * 512],
                                lnw_tile[:, k:k + 1],
                                vs[ni][:],
                                Alu.mult, Alu.mult)

            # =========== phase 1+2: attention (needs hT) ===========
            with tc.tile_pool(name="residp", bufs=1) as residp:
                with tc.tile_pool(name="hp", bufs=1) as hp:
                    hT = hp.tile([P, KT, T], f32)
                    nc.sync.dma_start(hT[:], hT_d.rearrange("(k p) t -> p k t", p=P))

                    # ---- ln1 + qkv + attention ----
                    with tc.tile_pool(name="p1", bufs=2) as p1:
                        qkvT = p1.tile([P, 2, T], f32r, bufs=1)
                        with tc.tile_pool(name="p1a", bufs=1) as p1a:
                            hnT = p1a.tile([P, KT, T], f32r)
                            rmsnorm(hT, ln1w, hnT)

                            with tc.tile_pool(name="ps1", bufs=1, space="PSUM") as ps1:
                                qkvw = p1a.tile([P, KT, 256], f32r)
                                nc.sync.dma_start(
                                    qkvw[:], qkvwT_d.rearrange("(k p) m -> p k m", p=P))
                                for mi in range(2):
                                    for ni in range(2):
                                        pq = ps1.tile([P, 512], f32, tag="pqkv", bufs=2)
                                        for k in range(KT):
                                            nc.tensor.matmul(
                                                pq[:], qkvw[:, k, mi * P:(mi + 1) * P],
                                                hnT[:, k, ni * 512:(ni + 1) * 512],
                                                start=(k == 0), stop=(k == KT - 1))
                                        nc.vector.tensor_copy(
                                            qkvT[:, mi, ni * 512:(ni + 1) * 512], pq[:])

                        # v to token-major [128tok, 8tiles, 64]
                        v_sb = p1.tile([P, KT, HD], f32r, bufs=1)
                        with tc.tile_pool(name="ps1v", bufs=1, space="PSUM") as ps1v:
                            for ti in range(KT):
                                pvt = ps1v.tile([P, HD], f32, tag="pvt", bufs=2)
                                nc.tensor.transpose(
                                    pvt[:],
                                    qkvT[HD:P, 1, ti * P:(ti + 1) * P].bitcast(f32),
                                    ident[HD:P, HD:P])
                                nc.vector.tensor_copy(v_sb[:, ti, :], pvt[:])

                        attn_sb = pers.tile([HD, 2, T], f32)
                        am = p1.tile([P, len(OFFS), 512], mybir.dt.bfloat16, bufs=1)
                        nc.sync.dma_start(am[:], amask_d.rearrange("o p f -> p o f"))

                        # re-base head-1 q to partitions 0..63 (SBUF->SBUF DMA)
                        q1_sb = p1.tile([HD, T], f32r, bufs=1)
                        nc.sync.dma_start(q1_sb[:], qkvT[HD:P, 0, :])

                        with tc.tile_pool(name="ps1b", bufs=1, space="PSUM") as ps1b:
                            for h in range(2):
                                qT = qkvT[0:HD, 0, :] if h == 0 else q1_sb[:]
                                kTT = qkvT[0:HD, 1, :]
                                for qt in range(2):
                                    kis = QT_KIS[qt]
                                    ppv = ps1b.tile([HD, 512], f32, tag="ppv")
                                    pcs = ps1b.tile([1, 512], f32, tag="pcs")
                                    for idx, ki in enumerate(kis):
                                        pscore = ps1b.tile([P, 512], f32,
                                                           tag="pscore", bufs=2)
                                        nc.tensor.matmul(
                                            pscore[:], kTT[:, ki * P:(ki + 1) * P],
                                            qT[:, qt * 512:(qt + 1) * 512],
                                            start=True, stop=True)
                                        off_i = OFFS.index(qt * 512 - ki * P)
                                        sm = p1.tile([P, 512], f32, tag="sm")
                                        nc.vector.scalar_tensor_tensor(
                                            sm[:], pscore[:], SCALE,
                                            am[:, off_i, :], Alu.mult, Alu.add)
                                        pexp = p1.tile([P, 512], f32r, tag="pexp")
                                        nc.scalar.activation(pexp[:], sm[:], Act.Exp)
                                        nc.tensor.matmul(
                                            pcs[:], ones128[:], pexp[:],
                                            start=(idx == 0),
                                            stop=(idx == len(kis) - 1))
                                        nc.tensor.matmul(
                                            ppv[:], v_sb[:, ki, :], pexp[:],
                                            start=(idx == 0),
                                            stop=(idx == len(kis) - 1))
                                    inv = p1.tile([1, 512], f32r, tag="inv")
                                    nc.vector.reciprocal(inv[:], pcs[:])
                                    pbc = ps1b.tile([P, 512], f32, tag="pbc2")
                                    nc.tensor.matmul(pbc[:], ones1r[:], inv[:],
                                                     start=True, stop=True)
                                    binv = p1.tile([HD, 512], f32, tag="binv")
                                    nc.vector.tensor_copy(binv[:], pbc[:HD, :])
                                    nc.vector.tensor_tensor(
                                        attn_sb[:, h, qt * 512:(qt + 1) * 512],
                                        ppv[:], binv[:], Alu.mult)

                        nc.sync.dma_start(
                            ag1_in[:].rearrange("(h d) t -> d h t", h=2), attn_sb[:])
                        if not profile:
                            nc.gpsimd.collective_compute(
                                "AllGather", Alu.bypass, replica_groups=rg,
                                ins=[ag1_in[:]], outs=[ag1_out[:]])

                    # ---- o-proj (fp32) + AG2 + residual ----
                    with tc.tile_pool(name="p2", bufs=2) as p2, \
                         tc.tile_pool(name="ps2", bufs=2, space="PSUM") as ps2:
                        ow = p2.tile([P, KT, P], f32, bufs=1)
                        nc.sync.dma_start(
                            ow[:], owT_d.rearrange("(k p) m -> p k m", p=P))
                        af = p2.tile([P, KT, T], f32, bufs=1)
                        nc.sync.dma_start(
                            af[:], ag1_out.rearrange("(k p) t -> p k t", p=P))
                        ao_sl = p2.tile([P, T], f32, bufs=1)
                        for ni in range(2):
                            po = ps2.tile([P, 512], f32, tag="po")
                            for k in range(KT):
                                nc.tensor.matmul(
                                    po[:], ow[:, k, :],
                                    af[:, k, ni * 512:(ni + 1) * 512],
                                    start=(k == 0), stop=(k == KT - 1))
                            nc.vector.tensor_copy(
                                ao_sl[:, ni * 512:(ni + 1) * 512], po[:])
                        nc.sync.dma_start(ag2_in[:], ao_sl[:])
                        if not profile:
                            nc.gpsimd.collective_compute(
                                "AllGather", Alu.bypass, replica_groups=rg,
                                ins=[ag2_in[:]], outs=[ag2_out[:]])

                        residT = residp.tile([P, KT, T], f32)
                        nc.sync.dma_start(
                            residT[:], ag2_out.rearrange("(k p) t -> p k t", p=P))
                        for k in range(KT):
                            nc.vector.tensor_add(residT[:, k, :], hT[:, k, :],
                                                 residT[:, k, :])
                        nc.sync.dma_start(
                            residT_d.rearrange("(k p) t -> p k t", p=P), residT[:])
                # hT pool closed here

                # =========== phase 3: ln2 + router + top2 weights ===========
                h2T = pers.tile([P, KT, T], f32r)
                wb = pers.tile([P, T], f32)
                rmsnorm(residT, ln2w, h2T)

                with tc.tile_pool(name="p3", bufs=2) as p3, \
                     tc.tile_pool(name="ps3", bufs=1, space="PSUM") as ps3:
                    rw = p3.tile([P, KT, E], f32r, bufs=1)
                    nc.sync.dma_start(rw[:], rwT_d[:])
                    logT = p3.tile([E, T], f32, bufs=1)
                    for ni in range(2):
                        pr = ps3.tile([E, 512], f32, tag="pr", bufs=2)
                        for k in range(KT):
                            nc.tensor.matmul(pr[:], rw[:, k, :],
                                             h2T[:, k, ni * 512:(ni + 1) * 512],
                                             start=(k == 0), stop=(k == KT - 1))
                        nc.vector.tensor_copy(logT[:, ni * 512:(ni + 1) * 512],
                                              pr[:])

                    wT = p3.tile([E, T], f32r, bufs=1)
                    for ti in range(KT):
                        ptr = ps3.tile([P, E], f32, tag="ptr", bufs=2)
                        nc.tensor.transpose(ptr[:], logT[:, ti * P:(ti + 1) * P],
                                            ident[:E, :E])
                        lg = p3.tile([P, E], f32, tag="lg")
                        nc.vector.tensor_copy(lg[:], ptr[:])
                        m1 = p3.tile([P, 1], f32, tag="m1")
                        nc.vector.reduce_max(m1[:], lg[:], axis=AxX)
                        nm1 = p3.tile([P, 1], f32, tag="nm1")
                        nc.vector.tensor_scalar_mul(nm1[:], m1[:], -1.0)
                        ex = p3.tile([P, E], f32, tag="ex")
                        nc.scalar.activation(ex[:], lg[:], Act.Exp, bias=nm1[:])
                        den = p3.tile([P, 1], f32, tag="den")
                        nc.vector.reduce_sum(den[:], ex[:], axis=AxX)
                        inv2 = p3.tile([P, 1], f32, tag="inv2")
                        nc.vector.reciprocal(inv2[:], den[:])
                        eq = p3.tile([P, E], f32, tag="eq")
                        nc.vector.tensor_scalar(eq[:], lg[:], m1[:], None,
                                                Alu.is_equal)
                        msk = p3.tile([P, E], f32, tag="msk")
                        nc.vector.scalar_tensor_tensor(msk[:], eq[:], NEG, lg[:],
                                                       Alu.mult, Alu.add)
                        m2 = p3.tile([P, 1], f32, tag="m2")
                        nc.vector.reduce_max(m2[:], msk[:], axis=AxX)
                        sel = p3.tile([P, E], f32, tag="sel")
                        nc.vector.tensor_scalar(sel[:], lg[:], m2[:], None,
                                                Alu.is_ge)
                        wtm = p3.tile([P, E], f32, tag="wtm")
                        nc.vector.tensor_scalar_mul(wtm[:], ex[:], inv2[:])
                        nc.vector.tensor_tensor(wtm[:], wtm[:], sel[:], Alu.mult)
                        pwt = ps3.tile([E, P], f32, tag="pwt", bufs=2)
                        nc.tensor.transpose(pwt[:], wtm[:], ident[:])
                        nc.vector.tensor_copy(wT[:, ti * P:(ti + 1) * P], pwt[:])

                    wrow = p3.tile([1, T], f32r, bufs=1)
                    for ni in range(2):
                        pwr = ps3.tile([1, 512], f32, tag="pwr")
                        nc.tensor.matmul(pwr[:], onehot[:],
                                         wT[:, ni * 512:(ni + 1) * 512],
                                         start=True, stop=True)
                        nc.vector.tensor_copy(wrow[:, ni * 512:(ni + 1) * 512],
                                              pwr[:])
                    for ni in range(2):
                        pwb = ps3.tile([P, 512], f32, tag="pwb")
                        nc.tensor.matmul(pwb[:], ones1r[:],
                                         wrow[:, ni * 512:(ni + 1) * 512],
                                         start=True, stop=True)
                        nc.vector.tensor_copy(wb[:, ni * 512:(ni + 1) * 512],
                                              pwb[:])
            # residT pool closed here

            # =========== phase 4: expert FFN (dense over T) ===========
            with tc.tile_pool(name="wpool", bufs=2) as wpool, \
                 tc.tile_pool(name="apool", bufs=1) as apool, \
                 tc.tile_pool(name="spool", bufs=2) as spool, \
                 tc.tile_pool(name="ps4", bufs=1, space="PSUM") as ps4:
                act = apool.tile([P, JT, T], f32r)
                for j in range(JT):
                    wg = wpool.tile([P, KT * P], f32r, tag="wg", bufs=2)
                    nc.sync.dma_start(wg[:], wsp_d[j])
                    wu = wpool.tile([P, KT * P], f32r, tag="wu", bufs=2)
                    nc.sync.dma_start(wu[:], wsp_d[JT + j])
                    for ni in range(2):
                        pg = ps4.tile([P, 512], f32, tag=f"pg{ni}")
                        pu = ps4.tile([P, 512], f32, tag=f"pu{ni}")
                        for k in range(KT):
                            nc.tensor.matmul(pg[:], wg[:, k * P:(k + 1) * P],
                                             h2T[:, k, ni * 512:(ni + 1) * 512],
                                             start=(k == 0), stop=(k == KT - 1))
                        for k in range(KT):
                            nc.tensor.matmul(pu[:], wu[:, k * P:(k + 1) * P],
                                             h2T[:, k, ni * 512:(ni + 1) * 512],
                                             start=(k == 0), stop=(k == KT - 1))
                        sil = spool.tile([P, 512], f32, tag="sil")
                        nc.scalar.activation(sil[:], pg[:], Act.Silu)
                        nc.vector.tensor_tensor(act[:, j, ni * 512:(ni + 1) * 512],
                                                sil[:], pu[:], Alu.mult)

                for m in range(MT):
                    w2 = wpool.tile([P, JT * P], f32r, tag="w2", bufs=2)
                    nc.sync.dma_start(w2[:], w2p_d[m])
                    for ni in range(2):
                        pd = ps4.tile([P, 512], f32, tag="pd", bufs=2)
                        for j in range(JT):
                            nc.tensor.matmul(pd[:], w2[:, j * P:(j + 1) * P],
                                             act[:, j, ni * 512:(ni + 1) * 512],
                                             start=(j == 0), stop=(j == JT - 1))
                        eo = spool.tile([P, 512], f32, tag="eo")
                        nc.vector.tensor_tensor(eo[:], pd[:],
                                                wb[:, ni * 512:(ni + 1) * 512],
                                                Alu.mult)
                        nc.sync.dma_start(rs_in[m * P:(m + 1) * P,
                                                ni * 512:(ni + 1) * 512], eo[:])

                if not profile:
                    nc.gpsimd.collective_compute(
                        "ReduceScatter", Alu.add, replica_groups=rg,
                        ins=[rs_in[:]], outs=[rs_out[:]])
                out_sb = spool.tile([P, T], f32, tag="osb")
                nc.sync.dma_start(out_sb[:], rs_out[:])
                nc.sync.dma_start(moe_sl_d[:], out_sb[:])

    nc.compile()
    return nc


_NC = None


def _get_nc():
    global _NC
    if _NC is None:
        _NC = _build()
    return _NC


def _pack_inputs(hidden_states, ln1_w, qkv_w, o_w, ln2_w, router_w, ws, w2s):
    hidden_states = np.asarray(hidden_states, np.float32)
    qkv_w = np.asarray(qkv_w, np.float32)
    o_w = np.asarray(o_w, np.float32)
    router_w = np.asarray(router_w, np.float32)
    ws = np.asarray(ws, np.float32)
    w2s = np.asarray(w2s, np.float32)
    ln1_w = np.asarray(ln1_w, np.float32)
    ln2_w = np.asarray(ln2_w, np.float32)

    hT = np.ascontiguousarray(hidden_states.T)
    ln1p = np.ascontiguousarray(ln1_w.reshape(KT, P).T)
    ln2p = np.ascontiguousarray(ln2_w.reshape(KT, P).T)
    rwT = np.ascontiguousarray(
        router_w.T.reshape(KT, P, E).transpose(1, 0, 2))

    amask = np.empty((len(OFFS), P, 512), np.float32)
    pp = np.arange(P)[:, None]
    ff = np.arange(512)[None, :]
    for i, off in enumerate(OFFS):
        d = off + ff - pp
        amask[i] = np.where((d >= 0) & (d < SW), 0.0, NEG)
    import ml_dtypes
    amask = amask.astype(ml_dtypes.bfloat16)

    ones128 = np.ones((P, 1), np.float32)
    ones1r = np.ones((1, P), np.float32)

    in_maps = []
    for c in range(NCORES):
        qrows = qkv_w[2 * c * HD:(2 * c + 2) * HD]
        krows = qkv_w[NH * HD + (c // 2) * HD: NH * HD + (c // 2 + 1) * HD]
        vrows = qkv_w[(NH + NKV) * HD + (c // 2) * HD:
                      (NH + NKV) * HD + (c // 2 + 1) * HD]
        qkv_sh = np.concatenate([qrows, krows, vrows], axis=0)   # [256, H]
        qkvwT = np.ascontiguousarray(qkv_sh.T)                   # [H, 256]
        owT = np.ascontiguousarray(o_w[c * P:(c + 1) * P, :].T)  # [NH*HD, 128]

        wsT = ws[c].T                                            # [H, 2I]
        wsp = np.ascontiguousarray(
            wsT.reshape(KT, P, 2 * JT, P).transpose(2, 1, 0, 3)
               .reshape(2 * JT, P, KT * P))
        w2T = w2s[c].T                                           # [I, H]
        w2p = np.ascontiguousarray(
            w2T.reshape(JT, P, MT, P).transpose(2, 1, 0, 3)
               .reshape(MT, P, JT * P))

        onehot = np.zeros((E, 1), np.float32)
        onehot[c, 0] = 1.0

        in_maps.append({
            "hT": hT, "qkvwT": qkvwT, "owT": owT,
            "ln1w": ln1p, "ln2w": ln2p, "rwT": rwT,
            "wsp": wsp, "w2p": w2p, "amask": amask,
            "onehot": onehot, "ones128": ones128, "ones1r": ones1r,
        })
    return in_maps


def kernel(hidden_states, positions, ln1_w, qkv_w, o_w, ln2_w, router_w, ws, w2s):
    nc = _get_nc()
    in_maps = _pack_inputs(hidden_states, ln1_w, qkv_w, o_w, ln2_w,
                           router_w, ws, w2s)
    res = run_bass_kernel_spmd(nc, in_maps, list(range(NCORES)))
    moe_T = np.concatenate([res.results[c]["moe_slice"] for c in range(NCORES)],
                           axis=0)                               # [H, T]
    moe_out = np.ascontiguousarray(moe_T.T)
    residual = np.ascontiguousarray(res.results[0]["residT"].T)
    return moe_out, residual



# revision 2
# speedup vs baseline: 18847.6602x; 18847.6602x over previous
"""Trainium2 Bass kernel for nn_JambaAttentionDecoderLayer (8-core SPMD).

Sharding: tensor-parallel attention (2 q-heads + 1 kv-head per core,
o-proj column-sharded, two AllGathers) + expert parallelism for the MoE
(1 expert per core, dense over tokens, ReduceScatter combine).

Everything on-device is computed in a feature-major ("transposed") layout
[feature, token] so every matmul contraction dim lands on SBUF partitions
without runtime transposes of activations.  Weights are transposed/packed
on the host while sharding.  Big matmuls run in float32r (full PE speed
for free-dim >= 256, ~2^-13 rounding).
"""

import numpy as np

import concourse.bass as bass
import concourse.tile as tile
import concourse.mybir as mybir
from concourse import bacc
from concourse.bass_utils import run_bass_kernel_spmd

# dims (hardcoded per spec)
T = 1024
H = 1024
NH = 16
NKV = 4
HD = 64
I = 2816
E = 8
SW = 512
EPS = 1e-6
SCALE = HD ** -0.5

NCORES = 8
P = 128
KT = H // P          # 8 k-tiles over H
JT = I // P          # 22 k-tiles over I
MT = H // P          # 8 m-tiles over H
NEG = -1.0e30

f32 = mybir.dt.float32
f32r = mybir.dt.float32r

# attention mask offsets: off = q_tile_start - k_tile_start for [128k,512q] tiles
OFFS = [-384, -256, -128, 0, 128, 256, 384, 512]
QT_KIS = {0: list(range(0, 4)), 1: list(range(0, 8))}

AxX = mybir.AxisListType.X
Alu = mybir.AluOpType
Act = mybir.ActivationFunctionType


def _build(profile=False):
    ndev = 1 if profile else NCORES
    nc = bacc.Bacc("TRN2", target_bir_lowering=False, debug=False,
                   num_devices=ndev)

    # ---- kernel I/O ----
    hT_d = nc.dram_tensor("hT", [H, T], f32, kind="ExternalInput")
    qkvwT_d = nc.dram_tensor("qkvwT", [H, 256], f32r, kind="ExternalInput")
    owT_d = nc.dram_tensor("owT", [H, P], f32, kind="ExternalInput")
    ln1w_d = nc.dram_tensor("ln1w", [P, KT], f32, kind="ExternalInput")
    ln2w_d = nc.dram_tensor("ln2w", [P, KT], f32, kind="ExternalInput")
    rwT_d = nc.dram_tensor("rwT", [P, KT, E], f32r, kind="ExternalInput")
    wsp_d = nc.dram_tensor("wsp", [2 * JT, P, KT * P], f32r, kind="ExternalInput")
    w2p_d = nc.dram_tensor("w2p", [MT, P, JT * P], f32r, kind="ExternalInput")
    amask_d = nc.dram_tensor("amask", [len(OFFS), P, 512], mybir.dt.bfloat16,
                             kind="ExternalInput")
    onehot_d = nc.dram_tensor("onehot", [E, 1], f32r, kind="ExternalInput")
    ones128_d = nc.dram_tensor("ones128", [P, 1], f32r, kind="ExternalInput")
    ones1r_d = nc.dram_tensor("ones1r", [1, P], f32r, kind="ExternalInput")

    moe_sl_d = nc.dram_tensor("moe_slice", [P, T], f32, kind="ExternalOutput")
    residT_d = nc.dram_tensor("residT", [H, T], f32, kind="ExternalOutput")

    rg = [list(range(NCORES))]

    import contextlib
    lp = getattr(nc, "allow_low_precision", None)
    lp_cm = lp(reason="float32r matmul operands; rounding ~2^-13 acceptable") \
        if lp else contextlib.nullcontext()
    with lp_cm, tile.TileContext(nc) as tc:
        with tc.tile_pool(name="const", bufs=1) as cpool, \
             tc.tile_pool(name="persist", bufs=1) as pers, \
             tc.tile_pool(name="dram", bufs=1, space="DRAM") as dram:

            # ---- constants ----
            ones128 = cpool.tile([P, 1], f32r)
            nc.sync.dma_start(ones128[:], ones128_d[:])
            ones1r = cpool.tile([1, P], f32r)
            nc.sync.dma_start(ones1r[:], ones1r_d[:])
            onehot = cpool.tile([E, 1], f32r)
            nc.sync.dma_start(onehot[:], onehot_d[:])
            ln1w = cpool.tile([P, KT], f32)
            nc.sync.dma_start(ln1w[:], ln1w_d[:])
            ln2w = cpool.tile([P, KT], f32)
            nc.sync.dma_start(ln2w[:], ln2w_d[:])
            ident = cpool.tile([P, P], f32)
            from concourse.masks import make_identity
            make_identity(nc, ident[:])

            # dram bounce buffers for collectives
            ag1_in = dram.tile([P, T], f32)
            ag1_out = dram.tile([H, T], f32, addr_space="Shared")
            ag2_in = dram.tile([P, T], f32)
            ag2_out = dram.tile([H, T], f32, addr_space="Shared")
            rs_in = dram.tile([H, T], f32)
            rs_out = dram.tile([P, T], f32)

            # =========== RMSNorm helper (feature-major) ===========
            def rmsnorm(src_tile, lnw_tile, dst_tile):
                with tc.tile_pool(name="rn", bufs=1) as tmp, \
                     tc.tile_pool(name="rnps", bufs=1, space="PSUM") as psum:
                    vs = [None, None]
                    for ni in range(2):
                        pv = psum.tile([1, 512], f32, tag="pvar")
                        for k in range(KT):
                            sq = tmp.tile([P, 512], f32r, tag="sq", bufs=2)
                            nc.scalar.activation(
                                sq[:], src_tile[:, k, ni * 512:(ni + 1) * 512],
                                Act.Square)
                            nc.tensor.matmul(pv[:], ones128[:], sq[:],
                                             start=(k == 0), stop=(k == KT - 1))
                        v = tmp.tile([1, 512], f32, tag="vv")
                        nc.vector.tensor_scalar(v[:], pv[:], 1.0 / H, EPS,
                                                Alu.mult, Alu.add)
                        sd = tmp.tile([1, 512], f32, tag="sd")
                        nc.scalar.activation(sd[:], v[:], Act.Sqrt)
                        s = tmp.tile([1, 512], f32r, tag="ss")
                        nc.vector.reciprocal(s[:], sd[:])
                        pb = psum.tile([P, 512], f32, tag="pbc", bufs=2)
                        nc.tensor.matmul(pb[:], ones1r[:], s[:],
                                         start=True, stop=True)
                        vs[ni] = pb
                    for ni in range(2):
                        for k in range(KT):
                            nc.vector.scalar_tensor_tensor(
                                dst_tile[:, k, ni * 512:(ni + 1) * 512],
                                src_tile[:, k, ni * 512:(ni + 1) * 512],
                                lnw_tile[:, k:k + 1],
                                vs[ni][:],
                                Alu.mult, Alu.mult)

            # =========== phase 1+2: attention (needs hT) ===========
            with tc.tile_pool(name="residp", bufs=1) as residp:
                with tc.tile_pool(name="hp", bufs=1) as hp:
                    hT = hp.tile([P, KT, T], f32)
                    nc.sync.dma_start(hT[:], hT_d.rearrange("(k p) t -> p k t", p=P))

                    # ---- ln1 + qkv + attention ----
                    with tc.tile_pool(name="p1", bufs=2) as p1:
                        qkvT = p1.tile([P, 2, T], f32r, bufs=1)
                        with tc.tile_pool(name="p1a", bufs=1) as p1a:
                            hnT = p1a.tile([P, KT, T], f32r)
                            rmsnorm(hT, ln1w, hnT)

                            with tc.tile_pool(name="ps1", bufs=1, space="PSUM") as ps1:
                                qkvw = p1a.tile([P, KT, 256], f32r)
                                nc.sync.dma_start(
                                    qkvw[:], qkvwT_d.rearrange("(k p) m -> p k m", p=P))
                                for mi in range(2):
                                    for ni in range(2):
                                        pq = ps1.tile([P, 512], f32, tag="pqkv", bufs=2)
                                        for k in range(KT):
                                            nc.tensor.matmul(
                                                pq[:], qkvw[:, k, mi * P:(mi + 1) * P],
                                                hnT[:, k, ni * 512:(ni + 1) * 512],
                                                start=(k == 0), stop=(k == KT - 1))
                                        nc.vector.tensor_copy(
                                            qkvT[:, mi, ni * 512:(ni + 1) * 512], pq[:])

                        # v to token-major [128tok, 8tiles, 64]
                        v_sb = p1.tile([P, KT, HD], f32r, bufs=1)
                        with tc.tile_pool(name="ps1v", bufs=1, space="PSUM") as ps1v:
                            for ti in range(KT):
                                pvt = ps1v.tile([P, HD], f32, tag="pvt", bufs=2)
                                nc.tensor.transpose(
                                    pvt[:],
                                    qkvT[HD:P, 1, ti * P:(ti + 1) * P].bitcast(f32),
                                    ident[HD:P, HD:P])
                                nc.vector.tensor_copy(v_sb[:, ti, :], pvt[:])

                        attn_sb = pers.tile([HD, 2, T], f32)
                        am = p1.tile([P, len(OFFS), 512], mybir.dt.bfloat16, bufs=1)
                        nc.sync.dma_start(am[:], amask_d.rearrange("o p f -> p o f"))

                        # re-base head-1 q to partitions 0..63 (SBUF->SBUF DMA)
                        q1_sb = p1.tile([HD, T], f32r, bufs=1)
                        nc.sync.dma_start(q1_sb[:], qkvT[HD:P, 0, :])

                        with tc.tile_pool(name="ps1b", bufs=1, space="PSUM") as ps1b:
                            for h in range(2):
                                qT = qkvT[0:HD, 0, :] if h == 0 else q1_sb[:]
                                kTT = qkvT[0:HD, 1, :]
                                for qt in range(2):
                                    kis = QT_KIS[qt]
                                    ppv = ps1b.tile([HD, 512], f32, tag="ppv")
                                    pcs = ps1b.tile([1, 512], f32, tag="pcs")
                                    for idx, ki in enumerate(kis):
                                        pscore = ps1b.tile([P, 512], f32,
                                                           tag="pscore", bufs=2)
                                        nc.tensor.matmul(
                                            pscore[:], kTT[:, ki * P:(ki + 1) * P],
                                            qT[:, qt * 512:(qt + 1) * 512],
                                            start=True, stop=True)
                                        off_i = OFFS.index(qt * 512 - ki * P)
                                        sm = p1.tile([P, 512], f32, tag="sm")
                                        nc.vector.scalar_tensor_tensor(
                                            sm[:], pscore[:], SCALE,
                                            am[:, off_i, :], Alu.mult, Alu.add)
                                        pexp = p1.tile([P, 512], f32r, tag="pexp")
                                        nc.scalar.activation(pexp[:], sm[:], Act.Exp)
                                        nc.tensor.matmul(
                                            pcs[:], ones128[:], pexp[:],
                                            start=(idx == 0),
                                            stop=(idx == len(kis) - 1))
                                        nc.tensor.matmul(
                                            ppv[:], v_sb[:, ki, :], pexp[:],
                                            start=(idx == 0),
                                            stop=(idx == len(kis) - 1))
                                    inv = p1.tile([1, 512], f32r, tag="inv")
                                    nc.vector.reciprocal(inv[:], pcs[:])
                                    pbc = ps1b.tile([P, 512], f32, tag="pbc2")
                                    nc.tensor.matmul(pbc[:], ones1r[:], inv[:],
                                                     start=True, stop=True)
                                    binv = p1.tile([HD, 512], f32, tag="binv")
                                    nc.vector.tensor_copy(binv[:], pbc[:HD, :])
                                    nc.vector.tensor_tensor(
                                        attn_sb[:, h, qt * 512:(qt + 1) * 512],
                                        ppv[:], binv[:], Alu.mult)

                        nc.sync.dma_start(
                            ag1_in[:].rearrange("(h d) t -> d h t", h=2), attn_sb[:])
                        if not profile:
                            nc.gpsimd.collective_compute(
                                "AllGather", Alu.bypass, replica_groups=rg,
                                ins=[ag1_in[:]], outs=[ag1_out[:]])

                    # ---- o-proj (fp32) + AG2 + residual ----
                    with tc.tile_pool(name="p2", bufs=2) as p2, \
                         tc.tile_pool(name="ps2", bufs=2, space="PSUM") as ps2:
                        ow = p2.tile([P, KT, P], f32, bufs=1)
                        nc.sync.dma_start(
                            ow[:], owT_d.rearrange("(k p) m -> p k m", p=P))
                        af = p2.tile([P, KT, T], f32, bufs=1)
                        nc.sync.dma_start(
                            af[:], ag1_out.rearrange("(k p) t -> p k t", p=P))
                        ao_sl = p2.tile([P, T], f32, bufs=1)
                        for ni in range(2):
                            po = ps2.tile([P, 512], f32, tag="po")
                            for k in range(KT):
                                nc.tensor.matmul(
                                    po[:], ow[:, k, :],
                                    af[:, k, ni * 512:(ni + 1) * 512],
                                    start=(k == 0), stop=(k == KT - 1))
                            nc.vector.tensor_copy(
                                ao_sl[:, ni * 512:(ni + 1) * 512], po[:])
                        nc.sync.dma_start(ag2_in[:], ao_sl[:])
                        if not profile:
                            nc.gpsimd.collective_compute(
                                "AllGather", Alu.bypass, replica_groups=rg,
                                ins=[ag2_in[:]], outs=[ag2_out[:]])

                        residT = residp.tile([P, KT, T], f32)
                        nc.sync.dma_start(
                            residT[:], ag2_out.rearrange("(k p) t -> p k t", p=P))
                        for k in range(KT):
                            nc.vector.tensor_add(residT[:, k, :], hT[:, k, :],
                                                 residT[:, k, :])
                        nc.sync.dma_start(
                            residT_d.rearrange("(k p) t -> p k t", p=P), residT[:])
                # hT pool closed here

                # =========== phase 3: ln2 + router + top2 weights ===========
                h2T = pers.tile([P, KT, T], f32r)
                wb = pers.tile([P, T], f32)
                rmsnorm(residT, ln2w, h2T)

                with tc.tile_pool(name="p3", bufs=2) as p3, \
                     tc.tile_pool(name="ps3", bufs=1, space="PSUM") as ps3:
                    rw = p3.tile([P, KT, E], f32r, bufs=1)
                    nc.sync.dma_start(rw[:], rwT_d[:])
                    logT = p3.tile([E, T], f32, bufs=1)
                    for ni in range(2):
                        pr = ps3.tile([E, 512], f32, tag="pr", bufs=2)
                        for k in range(KT):
                            nc.tensor.matmul(pr[:], rw[:, k, :],
                                             h2T[:, k, ni * 512:(ni + 1) * 512],
                                             start=(k == 0), stop=(k == KT - 1))
                        nc.vector.tensor_copy(logT[:, ni * 512:(ni + 1) * 512],
                                              pr[:])

                    wT = p3.tile([E, T], f32r, bufs=1)
                    for ti in range(KT):
                        ptr = ps3.tile([P, E], f32, tag="ptr", bufs=2)
                        nc.tensor.transpose(ptr[:], logT[:, ti * P:(ti + 1) * P],
                                            ident[:E, :E])
                        lg = p3.tile([P, E], f32, tag="lg")
                        nc.vector.tensor_copy(lg[:], ptr[:])
                        m1 = p3.tile([P, 1], f32, tag="m1")
                        nc.vector.reduce_max(m1[:], lg[:], axis=AxX)
                        nm1 = p3.tile([P, 1], f32, tag="nm1")
                        nc.vector.tensor_scalar_mul(nm1[:], m1[:], -1.0)
                        ex = p3.tile([P, E], f32, tag="ex")
                        nc.scalar.activation(ex[:], lg[:], Act.Exp, bias=nm1[:])
                        den = p3.tile([P, 1], f32, tag="den")
                        nc.vector.reduce_sum(den[:], ex[:], axis=AxX)
                        inv2 = p3.tile([P, 1], f32, tag="inv2")
                        nc.vector.reciprocal(inv2[:], den[:])
                        eq = p3.tile([P, E], f32, tag="eq")
                        nc.vector.tensor_scalar(eq[:], lg[:], m1[:], None,
                                                Alu.is_equal)
                        msk = p3.tile([P, E], f32, tag="msk")
                        nc.vector.scalar_tensor_tensor(msk[:], eq[:], NEG, lg[:],
                                                       Alu.mult, Alu.add)
                        m2 = p3.tile([P, 1], f32, tag="m2")
                        nc.vector.reduce_max(m2[:], msk[:], axis=AxX)
                        sel = p3.tile([P, E], f32, tag="sel")
                        nc.vector.tensor_scalar(sel[:], lg[:], m2[:], None,
                                                Alu.is_ge)
                        wtm = p3.tile([P, E], f32, tag="wtm")
                        nc.vector.tensor_scalar_mul(wtm[:], ex[:], inv2[:])
                        nc.vector.tensor_tensor(wtm[:], wtm[:], sel[:], Alu.mult)
                        pwt = ps3.tile([E, P], f32, tag="pwt", bufs=2)
                        nc.tensor.transpose(pwt[:], wtm[:], ident[:])
                        nc.vector.tensor_copy(wT[:, ti * P:(ti + 1) * P], pwt[:])

                    wrow = p3.tile([1, T], f32r, bufs=1)
                    for ni in range(2):
                        pwr = ps3.tile([1, 512], f32, tag="pwr")
                        nc.tensor.matmul(pwr[:], onehot[:],
                                         wT[:, ni * 512:(ni + 1) * 512],
                                         start=True, stop=True)
                        nc.vector.tensor_copy(wrow[:, ni * 512:(ni + 1) * 512],
                                              pwr[:])
                    for ni in range(2):
                        pwb = ps3.tile([P, 512], f32, tag="pwb")
                        nc.tensor.matmul(pwb[:], ones1r[:],
                                         wrow[:, ni * 512:(ni + 1) * 512],
                                         start=True, stop=True)
                        nc.vector.tensor_copy(wb[:, ni * 512:(ni + 1) * 512],
                                              pwb[:])
            # residT pool closed here

            # =========== phase 4: expert FFN (dense over T) ===========
            with tc.tile_pool(name="wpool", bufs=2) as wpool, \
                 tc.tile_pool(name="apool", bufs=1) as apool, \
                 tc.tile_pool(name="spool", bufs=2) as spool, \
                 tc.tile_pool(name="ps4", bufs=1, space="PSUM") as ps4:
                act = apool.tile([P, JT, T], f32r)
                for j in range(JT):
                    wg = wpool.tile([P, KT * P], f32r, tag="wg", bufs=2)
                    nc.sync.dma_start(wg[:], wsp_d[j])
                    wu = wpool.tile([P, KT * P], f32r, tag="wu", bufs=2)
                    nc.sync.dma_start(wu[:], wsp_d[JT + j])
                    for ni in range(2):
                        pg = ps4.tile([P, 512], f32, tag=f"pg{ni}")
                        pu = ps4.tile([P, 512], f32, tag=f"pu{ni}")
                        for k in range(KT):
                            nc.tensor.matmul(pg[:], wg[:, k * P:(k + 1) * P],
                                             h2T[:, k, ni * 512:(ni + 1) * 512],
                                             start=(k == 0), stop=(k == KT - 1))
                        for k in range(KT):
                            nc.tensor.matmul(pu[:], wu[:, k * P:(k + 1) * P],
                                             h2T[:, k, ni * 512:(ni + 1) * 512],
                                             start=(k == 0), stop=(k == KT - 1))
                        sil = spool.tile([P, 512], f32, tag="sil")
                        nc.scalar.activation(sil[:], pg[:], Act.Silu)
                        nc.vector.tensor_tensor(act[:, j, ni * 512:(ni + 1) * 512],
                                                sil[:], pu[:], Alu.mult)

                for m in range(MT):
                    w2 = wpool.tile([P, JT * P], f32r, tag="w2", bufs=2)
                    nc.sync.dma_start(w2[:], w2p_d[m])
                    for ni in range(2):
                        pd = ps4.tile([P, 512], f32, tag="pd", bufs=2)
                        for j in range(JT):
                            nc.tensor.matmul(pd[:], w2[:, j * P:(j + 1) * P],
                                             act[:, j, ni * 512:(ni + 1) * 512],
                                             start=(j == 0), stop=(j == JT - 1))
                        eo = spool.tile([P, 512], f32, tag="eo")
                        nc.vector.tensor_tensor(eo[:], pd[:],
                                                wb[:, ni * 512:(ni + 1) * 512],
                                                Alu.mult)
                        nc.sync.dma_start(rs_in[m * P:(m + 1) * P,
                                                ni * 512:(ni + 1) * 512], eo[:])

                if not profile:
                    nc.gpsimd.collective_compute(
                        "ReduceScatter", Alu.add, replica_groups=rg,
                        ins=[rs_in[:]], outs=[rs_out[:]])
                out_sb = spool.tile([P, T], f32, tag="osb")
                nc.sync.dma_start(out_sb[:], rs_out[:])
                nc.sync.dma_start(moe_sl_d[:], out_sb[:])

    nc.compile()
    return nc


_NC = None


def _get_nc():
    global _NC
    if _NC is None:
        _NC = _build()
    return _NC


def _pack_inputs(hidden_states, ln1_w, qkv_w, o_w, ln2_w, router_w, ws, w2s):
    hidden_states = np.asarray(hidden_states, np.float32)
    qkv_w = np.asarray(qkv_w, np.float32)
    o_w = np.asarray(o_w, np.float32)
    router_w = np.asarray(router_w, np.float32)
    ws = np.asarray(ws, np.float32)
    w2s = np.asarray(w2s, np.float32)
    ln1_w = np.asarray(ln1_w, np.float32)
    ln2_w = np.asarray(ln2_w, np.float32)

    hT = np.ascontiguousarray(hidden_states.T)
    ln1p = np.ascontiguousarray(ln1_w.reshape(KT, P).T)
    ln2p = np.ascontiguousarray(ln2_w.reshape(KT, P).T)
    rwT = np.ascontiguousarray(
        router_w.T.reshape(KT, P, E).transpose(1, 0, 2))

    amask = np.empty((len(OFFS), P, 512), np.float32)
    pp = np.arange(P)[:, None]
    ff = np.arange(512)[None, :]
    for i, off in enumerate(OFFS):
        d = off + ff - pp
        amask[i] = np.where((d >= 0) & (d < SW), 0.0, NEG)
    import ml_dtypes
    amask = amask.astype(ml_dtypes.bfloat16)

    ones128 = np.ones((P, 1), np.float32)
    ones1r = np.ones((1, P), np.float32)

    in_maps = []
    for c in range(NCORES):
        qrows = qkv_w[2 * c * HD:(2 * c + 2) * HD]
        krows = qkv_w[NH * HD + (c // 2) * HD: NH * HD + (c // 2 + 1) * HD]
        vrows = qkv_w[(NH + NKV) * HD + (c // 2) * HD:
                      (NH + NKV) * HD + (c // 2 + 1) * HD]
        qkv_sh = np.concatenate([qrows, krows, vrows], axis=0)   # [256, H]
        qkvwT = np.ascontiguousarray(qkv_sh.T)                   # [H, 256]
        owT = np.ascontiguousarray(o_w[c * P:(c + 1) * P, :].T)  # [NH*HD, 128]

        wsT = ws[c].T                                            # [H, 2I]
        wsp = np.ascontiguousarray(
            wsT.reshape(KT, P, 2 * JT, P).transpose(2, 1, 0, 3)
               .reshape(2 * JT, P, KT * P))
        w2T = w2s[c].T                                           # [I, H]
        w2p = np.ascontiguousarray(
            w2T.reshape(JT, P, MT, P).transpose(2, 1, 0, 3)
               .reshape(MT, P, JT * P))

        onehot = np.zeros((E, 1), np.float32)
        onehot[c, 0] = 1.0

        in_maps.append({
            "hT": hT, "qkvwT": qkvwT, "owT": owT,
            "ln1w": ln1p, "ln2w": ln2p, "rwT": rwT,
            "wsp": wsp, "w2p": w2p, "amask": amask,
            "onehot": onehot, "ones128": ones128, "ones1r": ones1r,
        })
    return in_maps


LAST_RESULT = None


def kernel(hidden_states, positions, ln1_w, qkv_w, o_w, ln2_w, router_w, ws, w2s):
    global LAST_RESULT
    nc = _get_nc()
    in_maps = _pack_inputs(hidden_states, ln1_w, qkv_w, o_w, ln2_w,
                           router_w, ws, w2s)
    res = run_bass_kernel_spmd(nc, in_maps, list(range(NCORES)))
    LAST_RESULT = res
    moe_T = np.concatenate([res.results[c]["moe_slice"] for c in range(NCORES)],
                           axis=0)                               # [H, T]
    moe_out = np.ascontiguousarray(moe_T.T)
    residual = np.ascontiguousarray(res.results[0]["residT"].T)
    return moe_out, residual

